# revision 5
# baseline (speedup 1.0000x reference)
"""MultiHeadAttention Trainium2 kernel, tensor-parallel over heads on 8 NeuronCores.

Contract: kernel(**inputs) takes FULL inputs (x, Wq, Wk, Wv, Wo, bo) and
returns the FULL outputs (y, att) matching reference.py.

Sharding: 16 heads / 8 cores = 2 heads per core. Wq/Wk/Wv are sliced
column-wise by head (rows of the torch-Linear weight), Wo row-wise.
Per-core partial y outputs are summed on the host; att head-shards are
concatenated on the host.

Per-core dataflow (all matmuls in float32r: fp32 storage, ~1e-4 rel err,
full PE speed):
  stage 1: qT/kT/vT [128ch, 4096tok] = W_cT.T @ xT   (x.T uploaded by host)
  stage 2: vT -> v_aug [tok, (v_h0|1|v_h1|1)] via PE transposes
  pass A (per head, [k,q] layout): weiT = kT.T@qT -> expT (ACT) ->
          yT[65,q] += v_aug.T @ expT  (row 64 = softmax denominator S)
  pass B ([q,k] layout): wei = qT.T@kT -> att = exp(wei - lnS) via ACT
          per-partition bias -> DMA straight to DRAM (normalized).
  stage C: ycT *= 1/S (K=2 mask-broadcast matmul + DVE mul),
          y_partial = ycT.T @ WoT -> DRAM.
"""

import numpy as np

import concourse.bacc as bacc
import concourse.tile as tile
from concourse import mybir
from concourse.bass import ts
from concourse.bass_utils import run_bass_kernel_spmd
from concourse.masks import make_identity

F32 = mybir.dt.float32
F32R = mybir.dt.float32r
Exp = mybir.ActivationFunctionType.Exp
Ln = mybir.ActivationFunctionType.Ln

B, T, D, H = 2, 2048, 1024, 16
HS = D // H            # 64  head size
NCORES = 8
HPC = H // NCORES      # 2   heads per core
CH = HPC * HS          # 128 channels per core
BT = B * T             # 4096 tokens

LAST_EXEC_NS = None
LAST_RESULT = None
_NC_CACHE = None


def _build():
    nc = bacc.Bacc(None)

    xT = nc.dram_tensor("xT", [D, BT], F32R, kind="ExternalInput")
    wqT = nc.dram_tensor("wqT", [D, CH], F32R, kind="ExternalInput")
    wkT = nc.dram_tensor("wkT", [D, CH], F32R, kind="ExternalInput")
    wvT = nc.dram_tensor("wvT", [D, CH], F32R, kind="ExternalInput")
    woT = nc.dram_tensor("woT", [CH, D], F32R, kind="ExternalInput")
    hmask = nc.dram_tensor("hmask", [HPC, 128], F32R, kind="ExternalInput")
    att_out = nc.dram_tensor("att", [B, HPC, T, T], F32, kind="ExternalOutput")
    y_out = nc.dram_tensor("y", [BT, D], F32, kind="ExternalOutput")

    NKT = T // 128         # 16 k tiles per batch
    NQT = T // 128         # 16 q tiles per batch
    NBKT = BT // 128       # 32 k tiles total

    with tile.TileContext(nc) as tc:
        with tc.tile_pool(name="persist", bufs=1) as pp:
            qT = pp.tile([CH, BT], F32R, tag="qT")
            kT = pp.tile([CH, BT], F32R, tag="kT")
            wq_sb = pp.tile([128, 8, 128], F32R, tag="wq")
            wk_sb = pp.tile([128, 8, 128], F32R, tag="wk")
            wv_sb = pp.tile([128, 8, 128], F32R, tag="wv")
            wo_sb = pp.tile([CH, D], F32R, tag="wo")
            ident = pp.tile([128, 128], F32, tag="ident")
            hmask_sb = pp.tile([HPC, 128], F32R, tag="hmask")
            v_aug = pp.tile([128, NBKT, 2 * (HS + 1)], F32R, tag="v_aug")
            ycT = pp.tile([CH, BT], F32R, tag="ycT")
            S_sb = pp.tile([HPC, BT], F32, tag="S")      # [h, b*T + q]
            negLnS = pp.tile([128, NQT, B, HPC], F32, tag="negLnS")

            make_identity(nc, ident[:])
            nc.sync.dma_start(out=hmask_sb, in_=hmask[:, :])
            nc.sync.dma_start(out=wo_sb, in_=woT[:, :])
            for w_sb, w_dr in ((wq_sb, wqT), (wk_sb, wkT), (wv_sb, wvT)):
                nc.sync.dma_start(
                    out=w_sb[:],
                    in_=w_dr[:, :].rearrange("(dt p) c -> p dt c", p=128),
                )

            # ---------------- stage 1: QKV projections ----------------
            with (
                tc.tile_pool(name="vpool", bufs=1) as vp,
                tc.tile_pool(name="xp", bufs=1) as xp,
                tc.tile_pool(name="ps1", bufs=4, space="PSUM") as ps1,
                tc.tile_pool(name="pst", bufs=2, space="PSUM") as pst,
            ):
                vT = vp.tile([CH, BT], F32R, tag="vT")
                QTR = BT // 4  # 1024 tokens per quarter
                for quarter in range(4):
                    x_sb = xp.tile([128, 8, QTR], F32R, tag="xq")
                    for dt in range(8):
                        nc.sync.dma_start(
                            out=x_sb[:, dt, :],
                            in_=xT[dt * 128:(dt + 1) * 128,
                                   quarter * QTR:(quarter + 1) * QTR],
                        )
                    for tc4 in range(QTR // 512):
                        tok0 = quarter * QTR + tc4 * 512
                        for w_sb, dest in ((wq_sb, qT), (wk_sb, kT), (wv_sb, vT)):
                            acc = ps1.tile([128, 512], F32, tag="acc")
                            for dt in range(8):
                                nc.tensor.matmul(
                                    acc[:],
                                    w_sb[:, dt, :],
                                    x_sb[:, dt, ts(tc4, 512)],
                                    start=(dt == 0),
                                    stop=(dt == 7),
                                )
                            nc.vector.tensor_copy(dest[:, tok0:tok0 + 512], acc[:])

                # ---------------- stage 2: v_aug build ----------------
                ones_sb = pp.tile([128, NBKT], F32, tag="ones")
                nc.vector.memset(ones_sb[:], 1.0)
                nc.vector.tensor_copy(v_aug[:, :, HS], ones_sb[:])
                nc.vector.tensor_copy(v_aug[:, :, 2 * HS + 1], ones_sb[:])
                for bkt in range(NBKT):
                    tp = pst.tile([128, 128], F32, tag="tp")
                    nc.tensor.transpose(
                        tp[:], vT[:, ts(bkt, 128)].bitcast(F32), ident[:]
                    )
                    nc.vector.tensor_copy(v_aug[:, bkt, 0:HS], tp[:, 0:HS])
                    nc.vector.tensor_copy(
                        v_aug[:, bkt, HS + 1:2 * HS + 1], tp[:, HS:2 * HS]
                    )

            # ---------------- attention ----------------
            with (
                tc.tile_pool(name="psw", bufs=2, space="PSUM") as psw,
                tc.tile_pool(name="psy", bufs=1, space="PSUM") as psy,
                tc.tile_pool(name="pss", bufs=2, space="PSUM") as pss,
                tc.tile_pool(name="sbe", bufs=3) as sbe,
                tc.tile_pool(name="sba", bufs=3) as sba,
                tc.tile_pool(name="sbg", bufs=2) as sbg,
            ):
                for b in range(B):
                    # ----- pass A (both heads): unnormalized y + S -----
                    for h in range(HPC):
                        h0 = h * HS
                        for qc in range(2):  # 1024-wide q chunks
                            q0 = b * T + qc * 1024
                            yT_ps = psy.tile([HS + 1, 1024], F32, tag="yT")
                            for kt in range(NKT):
                                wei = psw.tile([128, 1024], F32, tag="wei")
                                for n in range(2):
                                    nc.tensor.matmul(
                                        wei[:, ts(n, 512)],
                                        kT[h0:h0 + HS, b * T + kt * 128:b * T + (kt + 1) * 128],
                                        qT[h0:h0 + HS, q0 + n * 512:q0 + (n + 1) * 512],
                                        start=True, stop=True,
                                    )
                                expT = sbe.tile([128, 1024], F32R, tag="expT")
                                nc.scalar.activation(expT[:], wei[:], Exp)
                                for n in range(2):
                                    nc.tensor.matmul(
                                        yT_ps[:, ts(n, 512)],
                                        v_aug[:, b * NKT + kt,
                                              h * (HS + 1):(h + 1) * (HS + 1)],
                                        expT[:, ts(n, 512)],
                                        start=(kt == 0), stop=(kt == NKT - 1),
                                    )
                            stage65 = sbg.tile([HS + 1, 1024], F32, tag="st65")
                            nc.vector.tensor_copy(stage65[:], yT_ps[:])
                            if h == 0:
                                nc.vector.tensor_copy(
                                    ycT[0:HS, q0:q0 + 1024], yT_ps[0:HS, :]
                                )
                            else:
                                nc.gpsimd.dma_start(
                                    out=ycT[HS:2 * HS, q0:q0 + 1024],
                                    in_=stage65[0:HS, :],
                                )
                            nc.gpsimd.dma_start(
                                out=S_sb[h:h + 1, q0:q0 + 1024],
                                in_=stage65[HS:HS + 1, :],
                            )

                    # ----- S -> -ln(S) transposed to [q-part, qt] -----
                    for qt in range(NQT):
                        stp = pss.tile([128, HPC], F32, tag="stp")
                        nc.tensor.transpose(
                            stp[:],
                            S_sb[0:HPC, b * T + qt * 128:b * T + (qt + 1) * 128],
                            ident[0:HPC, 0:HPC],
                        )
                        nc.scalar.activation(
                            negLnS[:, qt, b, :], stp[:], Ln
                        )
                    nc.vector.tensor_scalar_mul(
                        negLnS[:, :, b, :], negLnS[:, :, b, :], -1.0
                    )

                    # ----- pass B (both heads): normalized att -----
                    for h in range(HPC):
                        h0 = h * HS
                        for qt in range(NQT):
                            att_sb = sba.tile([128, T], F32, tag="att")
                            for kc in range(2):  # 1024-wide k chunks
                                weib = psw.tile([128, 1024], F32, tag="wei")
                                for n in range(2):
                                    nc.tensor.matmul(
                                        weib[:, ts(n, 512)],
                                        qT[h0:h0 + HS, b * T + qt * 128:b * T + (qt + 1) * 128],
                                        kT[h0:h0 + HS,
                                           b * T + kc * 1024 + n * 512:
                                           b * T + kc * 1024 + (n + 1) * 512],
                                        start=True, stop=True,
                                    )
                                nc.scalar.activation(
                                    att_sb[:, ts(kc, 1024)], weib[:], Exp,
                                    bias=negLnS[:, qt, b, h:h + 1],
                                )
                            nc.sync.dma_start(
                                out=att_out[b, h, ts(qt, 128), :],
                                in_=att_sb[:],
                            )

            # ---------------- stage C: scale + out-projection ----------------
            recipS = pp.tile([HPC, BT], F32, tag="recipS")
            recipSr = pp.tile([HPC, BT], F32R, tag="recipSr")
            nc.vector.reciprocal(recipS[:], S_sb[:])
            nc.vector.tensor_copy(recipSr[:], recipS[:])
            with (
                tc.tile_pool(name="psc", bufs=2, space="PSUM") as psc,
                tc.tile_pool(name="pso", bufs=4, space="PSUM") as pso,
                tc.tile_pool(name="sby", bufs=2) as sby,
            ):
                for c4 in range(4):  # 1024-token chunks
                    bc = psc.tile([128, 1024], F32, tag="bc")
                    for n in range(2):
                        nc.tensor.matmul(
                            bc[:, ts(n, 512)],
                            hmask_sb[:],
                            recipSr[0:HPC, c4 * 1024 + n * 512:c4 * 1024 + (n + 1) * 512],
                            start=True, stop=True,
                        )
                    nc.vector.tensor_mul(
                        ycT[:, ts(c4, 1024)], ycT[:, ts(c4, 1024)], bc[:]
                    )
                for tt in range(BT // 128):
                    y_sb = sby.tile([128, D], F32, tag="y")
                    for n in range(2):
                        op = pso.tile([128, 512], F32, tag="op")
                        nc.tensor.matmul(
                            op[:],
                            ycT[:, ts(tt, 128)],
                            wo_sb[:, ts(n, 512)],
                            start=True, stop=True,
                        )
                        nc.vector.tensor_copy(y_sb[:, ts(n, 512)], op[:])
                    nc.sync.dma_start(out=y_out[ts(tt, 128), :], in_=y_sb[:])

    nc.finalize()
    return nc


def kernel(x, Wq, Wk, Wv, Wo, bo, _trace=False, _tmpdir=None):
    global LAST_EXEC_NS, LAST_RESULT, _NC_CACHE
    x = np.asarray(x, dtype=np.float32)
    Wq = np.asarray(Wq, dtype=np.float32)
    Wk = np.asarray(Wk, dtype=np.float32)
    Wv = np.asarray(Wv, dtype=np.float32)
    Wo = np.asarray(Wo, dtype=np.float32)
    bo = np.asarray(bo, dtype=np.float32)

    scale = 1.0 / np.sqrt(np.float32(HS))
    xT_host = np.ascontiguousarray(x.reshape(BT, D).T)
    hm = np.zeros((HPC, 128), np.float32)
    for h in range(HPC):
        hm[h, h * HS:(h + 1) * HS] = 1.0

    in_maps = []
    for c in range(NCORES):
        cs = slice(c * CH, (c + 1) * CH)
        in_maps.append({
            "xT": xT_host,
            "wqT": np.ascontiguousarray(Wq[cs, :].T) * scale,
            "wkT": np.ascontiguousarray(Wk[cs, :].T),
            "wvT": np.ascontiguousarray(Wv[cs, :].T),
            "woT": np.ascontiguousarray(Wo[:, cs].T),
            "hmask": hm,
        })

    if _NC_CACHE is None:
        _NC_CACHE = _build()
    nc = _NC_CACHE

    res = run_bass_kernel_spmd(
        nc, in_maps, core_ids=list(range(NCORES)), trace=_trace, tmpdir=_tmpdir
    )
    LAST_EXEC_NS = res.exec_time_ns
    LAST_RESULT = res

    att = np.empty((B, H, T, T), dtype=np.float32)
    y = np.zeros((BT, D), dtype=np.float64)
    for c in range(NCORES):
        att[:, c * HPC:(c + 1) * HPC] = res.results[c]["att"]
        y += res.results[c]["y"]
    y = (y + bo).astype(np.float32).reshape(B, T, D)
    return y, att


# revision 6
# speedup vs baseline: 1.2080x; 1.2080x over previous
"""MultiHeadAttention Trainium2 kernel, tensor-parallel over heads on 8 NeuronCores.

Contract: kernel(**inputs) takes FULL inputs (x, Wq, Wk, Wv, Wo, bo) and
returns the FULL outputs (y, att) matching reference.py.

Sharding: 16 heads / 8 cores = 2 heads per core. Wq/Wk/Wv are sliced
column-wise by head (rows of the torch-Linear weight), Wo row-wise.
Per-core partial y outputs are summed on the host; att head-shards are
concatenated on the host.

Per-core dataflow (all matmuls in float32r: fp32 storage, ~1e-4 rel err,
full PE speed):
  stage 1: qT/kT/vT [128ch, 4096tok] = W_cT.T @ xT   (x.T uploaded by host)
  stage 2: vT -> v_aug [tok, (v_h0|1|v_h1|1)] via PE transposes
  pass A (per head, [k,q] layout): weiT = kT.T@qT -> expT (ACT) ->
          yT[65,q] += v_aug.T @ expT  (row 64 = softmax denominator S)
  pass B ([q,k] layout): wei = qT.T@kT -> att = exp(wei - lnS) via ACT
          per-partition bias -> DMA straight to DRAM (normalized).
  stage C: ycT *= 1/S (K=2 mask-broadcast matmul + DVE mul),
          y_partial = ycT.T @ WoT -> DRAM.
"""

import numpy as np

import concourse.bacc as bacc
import concourse.tile as tile
from concourse import mybir
from concourse.bass import ts
from concourse.bass_utils import run_bass_kernel_spmd
from concourse.masks import make_identity

F32 = mybir.dt.float32
F32R = mybir.dt.float32r
Exp = mybir.ActivationFunctionType.Exp
Ln = mybir.ActivationFunctionType.Ln

B, T, D, H = 2, 2048, 1024, 16
HS = D // H            # 64  head size
NCORES = 8
HPC = H // NCORES      # 2   heads per core
CH = HPC * HS          # 128 channels per core
BT = B * T             # 4096 tokens

LAST_EXEC_NS = None
LAST_RESULT = None
_NC_CACHE = None


def _build():
    nc = bacc.Bacc(None)

    xT = nc.dram_tensor("xT", [D, BT], F32R, kind="ExternalInput")
    wqT = nc.dram_tensor("wqT", [D, CH], F32R, kind="ExternalInput")
    wkT = nc.dram_tensor("wkT", [D, CH], F32R, kind="ExternalInput")
    wvT = nc.dram_tensor("wvT", [D, CH], F32R, kind="ExternalInput")
    woT = nc.dram_tensor("woT", [CH, D], F32R, kind="ExternalInput")
    hmask = nc.dram_tensor("hmask", [HPC, 128], F32R, kind="ExternalInput")
    att_out = nc.dram_tensor("att", [B, HPC, T, T], F32, kind="ExternalOutput")
    y_out = nc.dram_tensor("y", [BT, D], F32, kind="ExternalOutput")

    NKT = T // 128         # 16 k tiles per batch
    NQT = T // 128         # 16 q tiles per batch
    NBKT = BT // 128       # 32 k tiles total

    with tile.TileContext(nc) as tc:
        with tc.tile_pool(name="persist", bufs=1) as pp:
            qT = pp.tile([CH, BT], F32R, tag="qT")
            kT = pp.tile([CH, BT], F32R, tag="kT")
            wq_sb = pp.tile([128, 8, 128], F32R, tag="wq")
            wk_sb = pp.tile([128, 8, 128], F32R, tag="wk")
            wv_sb = pp.tile([128, 8, 128], F32R, tag="wv")
            wo_sb = pp.tile([CH, D], F32R, tag="wo")
            ident = pp.tile([128, 128], F32, tag="ident")
            hmask_sb = pp.tile([HPC, 128], F32R, tag="hmask")
            v_aug = pp.tile([128, NBKT, 2 * (HS + 1)], F32R, tag="v_aug")
            ycT = pp.tile([CH, BT], F32R, tag="ycT")
            S_sb = pp.tile([HPC, BT], F32, tag="S")      # [h, b*T + q]
            negLnS = pp.tile([128, NQT, B, HPC], F32, tag="negLnS")

            make_identity(nc, ident[:])
            nc.sync.dma_start(out=hmask_sb, in_=hmask[:, :])
            nc.sync.dma_start(out=wo_sb, in_=woT[:, :])
            for w_sb, w_dr in ((wq_sb, wqT), (wk_sb, wkT), (wv_sb, wvT)):
                nc.sync.dma_start(
                    out=w_sb[:],
                    in_=w_dr[:, :].rearrange("(dt p) c -> p dt c", p=128),
                )

            # ---------------- stage 1: QKV projections ----------------
            with (
                tc.tile_pool(name="vpool", bufs=1) as vp,
                tc.tile_pool(name="xp", bufs=1) as xp,
                tc.tile_pool(name="ps1", bufs=4, space="PSUM") as ps1,
                tc.tile_pool(name="pst", bufs=2, space="PSUM") as pst,
            ):
                vT = vp.tile([CH, BT], F32R, tag="vT")
                QTR = BT // 4  # 1024 tokens per quarter
                for quarter in range(4):
                    x_sb = xp.tile([128, 8, QTR], F32R, tag="xq")
                    for dt in range(8):
                        nc.sync.dma_start(
                            out=x_sb[:, dt, :],
                            in_=xT[dt * 128:(dt + 1) * 128,
                                   quarter * QTR:(quarter + 1) * QTR],
                        )
                    for tc4 in range(QTR // 512):
                        tok0 = quarter * QTR + tc4 * 512
                        for w_sb, dest in ((wq_sb, qT), (wk_sb, kT), (wv_sb, vT)):
                            acc = ps1.tile([128, 512], F32, tag="acc")
                            for dt in range(8):
                                nc.tensor.matmul(
                                    acc[:],
                                    w_sb[:, dt, :],
                                    x_sb[:, dt, ts(tc4, 512)],
                                    start=(dt == 0),
                                    stop=(dt == 7),
                                )
                            nc.vector.tensor_copy(dest[:, tok0:tok0 + 512], acc[:])

                # ---------------- stage 2: v_aug build ----------------
                ones_sb = pp.tile([128, NBKT], F32, tag="ones")
                nc.vector.memset(ones_sb[:], 1.0)
                nc.vector.tensor_copy(v_aug[:, :, HS], ones_sb[:])
                nc.vector.tensor_copy(v_aug[:, :, 2 * HS + 1], ones_sb[:])
                for bkt in range(NBKT):
                    tp = pst.tile([128, 128], F32, tag="tp")
                    nc.tensor.transpose(
                        tp[:], vT[:, ts(bkt, 128)].bitcast(F32), ident[:]
                    )
                    nc.vector.tensor_copy(v_aug[:, bkt, 0:HS], tp[:, 0:HS])
                    nc.vector.tensor_copy(
                        v_aug[:, bkt, HS + 1:2 * HS + 1], tp[:, HS:2 * HS]
                    )

            # ---------------- attention ----------------
            with (
                tc.tile_pool(name="psw", bufs=3, space="PSUM") as psw,
                tc.tile_pool(name="psy", bufs=1, space="PSUM") as psy,
                tc.tile_pool(name="sbe", bufs=3) as sbe,
                tc.tile_pool(name="sba", bufs=3) as sba,
                tc.tile_pool(name="sbg", bufs=2) as sbg,
                tc.tile_pool(name="sbr", bufs=1) as sbr,
            ):
                for b in range(B):
                    # ----- pass A (heads packed in PE row groups): y_unnorm + S -----
                    # wei/expT tiles are [128k, 1024] = [h0 512q | h1 512q]; the two
                    # heads' K=64 QKt matmuls overlap in disjoint PE row groups.
                    for qc in range(4):  # 512-wide q chunks
                        q0 = b * T + qc * 512
                        yT0 = psy.tile([HS + 1, 512], F32, tag="yT0")
                        yT1 = psy.tile([HS + 1, 512], F32, tag="yT1")
                        for kt in range(NKT):
                            k0 = b * T + kt * 128
                            wei = psw.tile([128, 1024], F32, tag="wei")
                            nc.tensor.matmul(
                                wei[:, 0:512],
                                kT[0:HS, k0:k0 + 128],
                                qT[0:HS, q0:q0 + 512],
                                start=True, stop=True,
                            )
                            nc.tensor.matmul(
                                wei[:, 512:1024],
                                kT[HS:2 * HS, k0:k0 + 128],
                                qT[HS:2 * HS, q0:q0 + 512],
                                start=True, stop=True,
                            )
                            expT = sbe.tile([128, 1024], F32R, tag="expT")
                            nc.scalar.activation(expT[:], wei[:], Exp)
                            nc.tensor.matmul(
                                yT0[:],
                                v_aug[:, b * NKT + kt, 0:HS + 1],
                                expT[:, 0:512],
                                start=(kt == 0), stop=(kt == NKT - 1),
                            )
                            nc.tensor.matmul(
                                yT1[:],
                                v_aug[:, b * NKT + kt, HS + 1:2 * (HS + 1)],
                                expT[:, 512:1024],
                                start=(kt == 0), stop=(kt == NKT - 1),
                            )
                        for h, yT_ps in ((0, yT0), (1, yT1)):
                            stage65 = sbg.tile([HS + 1, 512], F32, tag="st65")
                            nc.vector.tensor_copy(stage65[:], yT_ps[:])
                            if h == 0:
                                nc.vector.tensor_copy(
                                    ycT[0:HS, q0:q0 + 512], yT_ps[0:HS, :]
                                )
                            else:
                                nc.gpsimd.dma_start(
                                    out=ycT[HS:2 * HS, q0:q0 + 512],
                                    in_=stage65[0:HS, :],
                                )
                            nc.gpsimd.dma_start(
                                out=S_sb[h:h + 1, q0:q0 + 512],
                                in_=stage65[HS:HS + 1, :],
                            )

                    # ----- S -> -ln(S) transposed to [q-part, qt] -----
                    rawST = sbr.tile([128, NQT, HPC], F32, tag="rawST")
                    for qt in range(NQT):
                        stp = psw.tile([128, HPC], F32, tag="wei")
                        nc.tensor.transpose(
                            stp[:],
                            S_sb[0:HPC, b * T + qt * 128:b * T + (qt + 1) * 128],
                            ident[0:HPC, 0:HPC],
                        )
                        nc.vector.tensor_copy(rawST[:, qt, :], stp[:])
                    nc.scalar.activation(negLnS[:, :, b, :], rawST[:], Ln)
                    nc.vector.tensor_scalar_mul(
                        negLnS[:, :, b, :], negLnS[:, :, b, :], -1.0
                    )

                    # ----- pass B (heads pair-interleaved): normalized att -----
                    for qt in range(NQT):
                        qs = b * T + qt * 128
                        att0 = sba.tile([128, T], F32, tag="att")
                        att1 = sba.tile([128, T], F32, tag="att")
                        for kc in range(2):  # 1024-wide k chunks
                            wei0 = psw.tile([128, 1024], F32, tag="wei")
                            wei1 = psw.tile([128, 1024], F32, tag="wei")
                            for n in range(2):
                                ks = b * T + kc * 1024 + n * 512
                                nc.tensor.matmul(
                                    wei0[:, ts(n, 512)],
                                    qT[0:HS, qs:qs + 128],
                                    kT[0:HS, ks:ks + 512],
                                    start=True, stop=True,
                                )
                                nc.tensor.matmul(
                                    wei1[:, ts(n, 512)],
                                    qT[HS:2 * HS, qs:qs + 128],
                                    kT[HS:2 * HS, ks:ks + 512],
                                    start=True, stop=True,
                                )
                            nc.scalar.activation(
                                att0[:, ts(kc, 1024)], wei0[:], Exp,
                                bias=negLnS[:, qt, b, 0:1],
                            )
                            nc.scalar.activation(
                                att1[:, ts(kc, 1024)], wei1[:], Exp,
                                bias=negLnS[:, qt, b, 1:2],
                            )
                        nc.sync.dma_start(
                            out=att_out[b, 0, ts(qt, 128), :], in_=att0[:]
                        )
                        nc.sync.dma_start(
                            out=att_out[b, 1, ts(qt, 128), :], in_=att1[:]
                        )

            # ---------------- stage C: scale + out-projection ----------------
            recipS = pp.tile([HPC, BT], F32, tag="recipS")
            recipSr = pp.tile([HPC, BT], F32R, tag="recipSr")
            nc.vector.reciprocal(recipS[:], S_sb[:])
            nc.vector.tensor_copy(recipSr[:], recipS[:])
            with (
                tc.tile_pool(name="psc", bufs=2, space="PSUM") as psc,
                tc.tile_pool(name="pso", bufs=4, space="PSUM") as pso,
                tc.tile_pool(name="sby", bufs=2) as sby,
            ):
                for c4 in range(4):  # 1024-token chunks
                    bc = psc.tile([128, 1024], F32, tag="bc")
                    for n in range(2):
                        nc.tensor.matmul(
                            bc[:, ts(n, 512)],
                            hmask_sb[:],
                            recipSr[0:HPC, c4 * 1024 + n * 512:c4 * 1024 + (n + 1) * 512],
                            start=True, stop=True,
                        )
                    nc.vector.tensor_mul(
                        ycT[:, ts(c4, 1024)], ycT[:, ts(c4, 1024)], bc[:]
                    )
                for tt in range(BT // 128):
                    y_sb = sby.tile([128, D], F32, tag="y")
                    for n in range(2):
                        op = pso.tile([128, 512], F32, tag="op")
                        nc.tensor.matmul(
                            op[:],
                            ycT[:, ts(tt, 128)],
                            wo_sb[:, ts(n, 512)],
                            start=True, stop=True,
                        )
                        nc.vector.tensor_copy(y_sb[:, ts(n, 512)], op[:])
                    nc.sync.dma_start(out=y_out[ts(tt, 128), :], in_=y_sb[:])

    nc.finalize()
    return nc


def kernel(x, Wq, Wk, Wv, Wo, bo, _trace=False, _tmpdir=None):
    global LAST_EXEC_NS, LAST_RESULT, _NC_CACHE
    x = np.asarray(x, dtype=np.float32)
    Wq = np.asarray(Wq, dtype=np.float32)
    Wk = np.asarray(Wk, dtype=np.float32)
    Wv = np.asarray(Wv, dtype=np.float32)
    Wo = np.asarray(Wo, dtype=np.float32)
    bo = np.asarray(bo, dtype=np.float32)

    scale = 1.0 / np.sqrt(np.float32(HS))
    xT_host = np.ascontiguousarray(x.reshape(BT, D).T)
    hm = np.zeros((HPC, 128), np.float32)
    for h in range(HPC):
        hm[h, h * HS:(h + 1) * HS] = 1.0

    in_maps = []
    for c in range(NCORES):
        cs = slice(c * CH, (c + 1) * CH)
        in_maps.append({
            "xT": xT_host,
            "wqT": np.ascontiguousarray(Wq[cs, :].T) * scale,
            "wkT": np.ascontiguousarray(Wk[cs, :].T),
            "wvT": np.ascontiguousarray(Wv[cs, :].T),
            "woT": np.ascontiguousarray(Wo[:, cs].T),
            "hmask": hm,
        })

    if _NC_CACHE is None:
        _NC_CACHE = _build()
    nc = _NC_CACHE

    res = run_bass_kernel_spmd(
        nc, in_maps, core_ids=list(range(NCORES)), trace=_trace, tmpdir=_tmpdir
    )
    LAST_EXEC_NS = res.exec_time_ns
    LAST_RESULT = res

    att = np.empty((B, H, T, T), dtype=np.float32)
    y = np.zeros((BT, D), dtype=np.float64)
    for c in range(NCORES):
        att[:, c * HPC:(c + 1) * HPC] = res.results[c]["att"]
        y += res.results[c]["y"]
    y = (y + bo).astype(np.float32).reshape(B, T, D)
    return y, att


# revision 7
# speedup vs baseline: 1.3810x; 1.1432x over previous
"""MultiHeadAttention Trainium2 kernel, tensor-parallel over heads on 8 NeuronCores.

Contract: kernel(**inputs) takes FULL inputs (x, Wq, Wk, Wv, Wo, bo) and
returns the FULL outputs (y, att) matching reference.py.

Sharding: 16 heads / 8 cores = 2 heads per core. Wq/Wk/Wv are sliced
column-wise by head (rows of the torch-Linear weight), Wo row-wise.
Per-core partial y outputs are summed on the host; att head-shards are
concatenated on the host.

Per-core dataflow (all matmuls in float32r: fp32 storage, ~1e-4 rel err,
full PE speed):
  stage 1: qT/kT/vT [128ch, 4096tok] = W_cT.T @ xT   (x.T uploaded by host)
  stage 2: vT -> v_aug [tok, (v_h0|1|v_h1|1)] via PE transposes
  pass A (per head, [k,q] layout): weiT = kT.T@qT -> expT (ACT) ->
          yT[65,q] += v_aug.T @ expT  (row 64 = softmax denominator S)
  pass B ([q,k] layout): wei = qT.T@kT -> att = exp(wei - lnS) via ACT
          per-partition bias -> DMA straight to DRAM (normalized).
  stage C: ycT *= 1/S (K=2 mask-broadcast matmul + DVE mul),
          y_partial = ycT.T @ WoT -> DRAM.
"""

import numpy as np

import concourse.bacc as bacc
import concourse.tile as tile
from concourse import mybir
from concourse.bass import ts
from concourse.bass_utils import run_bass_kernel_spmd
from concourse.masks import make_identity

F32 = mybir.dt.float32
F32R = mybir.dt.float32r
Exp = mybir.ActivationFunctionType.Exp
Ln = mybir.ActivationFunctionType.Ln

B, T, D, H = 2, 2048, 1024, 16
HS = D // H            # 64  head size
NCORES = 8
HPC = H // NCORES      # 2   heads per core
CH = HPC * HS          # 128 channels per core
BT = B * T             # 4096 tokens

LAST_EXEC_NS = None
LAST_RESULT = None
_NC_CACHE = None


def _build():
    nc = bacc.Bacc(None)

    xT = nc.dram_tensor("xT", [D, BT], F32R, kind="ExternalInput")
    wqT = nc.dram_tensor("wqT", [D, CH], F32R, kind="ExternalInput")
    wkT = nc.dram_tensor("wkT", [D, CH], F32R, kind="ExternalInput")
    wvT = nc.dram_tensor("wvT", [D, CH], F32R, kind="ExternalInput")
    woT = nc.dram_tensor("woT", [CH, D], F32R, kind="ExternalInput")
    hmask = nc.dram_tensor("hmask", [HPC, 128], F32R, kind="ExternalInput")
    att_out = nc.dram_tensor("att", [B, HPC, T, T], F32, kind="ExternalOutput")
    y_out = nc.dram_tensor("y", [BT, D], F32, kind="ExternalOutput")

    NKT = T // 128         # 16 k tiles per batch
    NQT = T // 128         # 16 q tiles per batch
    NBKT = BT // 128       # 32 k tiles total

    with tile.TileContext(nc) as tc:
        with tc.tile_pool(name="persist", bufs=1) as pp:
            qT = pp.tile([CH, BT], F32R, tag="qT")
            kT = pp.tile([CH, BT], F32R, tag="kT")
            wq_sb = pp.tile([128, 8, 128], F32R, tag="wq")
            wk_sb = pp.tile([128, 8, 128], F32R, tag="wk")
            wv_sb = pp.tile([128, 8, 128], F32R, tag="wv")
            wo_sb = pp.tile([CH, D], F32R, tag="wo")
            ident = pp.tile([128, 128], F32, tag="ident")
            hmask_sb = pp.tile([HPC, 128], F32R, tag="hmask")
            v_aug = pp.tile([128, NBKT, 2 * (HS + 1)], F32R, tag="v_aug")
            ycT = pp.tile([CH, BT], F32R, tag="ycT")
            S_sb = pp.tile([HPC, BT], F32, tag="S")      # [h, b*T + q]
            negLnS = pp.tile([128, NQT, B, HPC], F32, tag="negLnS")

            make_identity(nc, ident[:])
            nc.sync.dma_start(out=hmask_sb, in_=hmask[:, :])
            nc.sync.dma_start(out=wo_sb, in_=woT[:, :])
            for w_sb, w_dr in ((wq_sb, wqT), (wk_sb, wkT), (wv_sb, wvT)):
                nc.sync.dma_start(
                    out=w_sb[:],
                    in_=w_dr[:, :].rearrange("(dt p) c -> p dt c", p=128),
                )

            # ---------------- stage 1: QKV projections ----------------
            with (
                tc.tile_pool(name="vpool", bufs=1) as vp,
                tc.tile_pool(name="xp", bufs=2) as xp,
                tc.tile_pool(name="ps1", bufs=4, space="PSUM") as ps1,
                tc.tile_pool(name="pst", bufs=2, space="PSUM") as pst,
            ):
                vT = vp.tile([CH, BT], F32R, tag="vT")
                QTR = BT // 4  # 1024 tokens per quarter
                for quarter in range(4):
                    x_sb = xp.tile([128, 8, QTR], F32R, tag="xq")
                    for dt in range(8):
                        nc.sync.dma_start(
                            out=x_sb[:, dt, :],
                            in_=xT[dt * 128:(dt + 1) * 128,
                                   quarter * QTR:(quarter + 1) * QTR],
                        )
                    for tc4 in range(QTR // 512):
                        tok0 = quarter * QTR + tc4 * 512
                        for w_sb, dest in ((wq_sb, qT), (wk_sb, kT), (wv_sb, vT)):
                            acc = ps1.tile([128, 512], F32, tag="acc")
                            for dt in range(8):
                                nc.tensor.matmul(
                                    acc[:],
                                    w_sb[:, dt, :],
                                    x_sb[:, dt, ts(tc4, 512)],
                                    start=(dt == 0),
                                    stop=(dt == 7),
                                )
                            nc.vector.tensor_copy(dest[:, tok0:tok0 + 512], acc[:])

                # ---------------- stage 2: v_aug build ----------------
                ones_sb = pp.tile([128, NBKT], F32, tag="ones")
                nc.vector.memset(ones_sb[:], 1.0)
                nc.vector.tensor_copy(v_aug[:, :, HS], ones_sb[:])
                nc.vector.tensor_copy(v_aug[:, :, 2 * HS + 1], ones_sb[:])
                for bkt in range(NBKT):
                    tp = pst.tile([128, 128], F32, tag="tp")
                    nc.tensor.transpose(
                        tp[:], vT[:, ts(bkt, 128)].bitcast(F32), ident[:]
                    )
                    nc.vector.tensor_copy(v_aug[:, bkt, 0:HS], tp[:, 0:HS])
                    nc.vector.tensor_copy(
                        v_aug[:, bkt, HS + 1:2 * HS + 1], tp[:, HS:2 * HS]
                    )

            # ---------------- attention ----------------
            with (
                tc.tile_pool(name="psw", bufs=3, space="PSUM") as psw,
                tc.tile_pool(name="psy", bufs=1, space="PSUM") as psy,
                tc.tile_pool(name="sbe", bufs=3) as sbe,
                tc.tile_pool(name="sba", bufs=3) as sba,
                tc.tile_pool(name="sbg", bufs=2) as sbg,
                tc.tile_pool(name="sbr", bufs=1) as sbr,
                tc.tile_pool(name="sby", bufs=3) as sby,
            ):
                for b in range(B):
                    # ----- pass A (heads packed in PE row groups): y_unnorm + S -----
                    # wei/expT tiles are [128k, 1024] = [h0 512q | h1 512q]; the two
                    # heads' K=64 QKt matmuls overlap in disjoint PE row groups.
                    for qc in range(4):  # 512-wide q chunks
                        q0 = b * T + qc * 512
                        yT0 = psy.tile([HS + 1, 512], F32, tag="yT0")
                        yT1 = psy.tile([HS + 1, 512], F32, tag="yT1")
                        for kt in range(NKT):
                            k0 = b * T + kt * 128
                            wei = psw.tile([128, 1024], F32, tag="wei")
                            nc.tensor.matmul(
                                wei[:, 0:512],
                                kT[0:HS, k0:k0 + 128],
                                qT[0:HS, q0:q0 + 512],
                                start=True, stop=True,
                            )
                            nc.tensor.matmul(
                                wei[:, 512:1024],
                                kT[HS:2 * HS, k0:k0 + 128],
                                qT[HS:2 * HS, q0:q0 + 512],
                                start=True, stop=True,
                            )
                            expT = sbe.tile([128, 1024], F32R, tag="expT")
                            nc.scalar.activation(expT[:], wei[:], Exp)
                            nc.tensor.matmul(
                                yT0[:],
                                v_aug[:, b * NKT + kt, 0:HS + 1],
                                expT[:, 0:512],
                                start=(kt == 0), stop=(kt == NKT - 1),
                            )
                            nc.tensor.matmul(
                                yT1[:],
                                v_aug[:, b * NKT + kt, HS + 1:2 * (HS + 1)],
                                expT[:, 512:1024],
                                start=(kt == 0), stop=(kt == NKT - 1),
                            )
                        for h, yT_ps in ((0, yT0), (1, yT1)):
                            stage65 = sbg.tile([HS + 1, 512], F32, tag="st65")
                            nc.vector.tensor_copy(stage65[:], yT_ps[:])
                            if h == 0:
                                nc.vector.tensor_copy(
                                    ycT[0:HS, q0:q0 + 512], yT_ps[0:HS, :]
                                )
                            else:
                                nc.gpsimd.dma_start(
                                    out=ycT[HS:2 * HS, q0:q0 + 512],
                                    in_=stage65[0:HS, :],
                                )
                            nc.gpsimd.dma_start(
                                out=S_sb[h:h + 1, q0:q0 + 512],
                                in_=stage65[HS:HS + 1, :],
                            )

                    # ----- S -> -ln(S) transposed to [q-part, qt] -----
                    rawST = sbr.tile([128, NQT, HPC], F32, tag="rawST")
                    for qt in range(NQT):
                        stp = psw.tile([128, HPC], F32, tag="wei")
                        nc.tensor.transpose(
                            stp[:],
                            S_sb[0:HPC, b * T + qt * 128:b * T + (qt + 1) * 128],
                            ident[0:HPC, 0:HPC],
                        )
                        nc.vector.tensor_copy(rawST[:, qt, :], stp[:])
                    nc.scalar.activation(negLnS[:, :, b, :], rawST[:], Ln)
                    nc.vector.tensor_scalar_mul(
                        negLnS[:, :, b, :], negLnS[:, :, b, :], -1.0
                    )

                    # ----- stage C for this b: 1/S scale + out-projection -----
                    # recipS = exp(-ln S) on ACT (avoids a 26us blocking DVE
                    # reciprocal); broadcast across head row-blocks via a K=2
                    # mask matmul, then project. Emitted here so the PE/DVE
                    # work overlaps pass B's ACT-bound stretch.
                    lnS_b = sbr.tile([HPC, T], F32, tag="lnSb")
                    nc.scalar.activation(
                        lnS_b[:], S_sb[0:HPC, b * T:(b + 1) * T], Ln
                    )
                    recipSr_b = sbr.tile([HPC, T], F32R, tag="recipSb")
                    nc.scalar.activation(recipSr_b[:], lnS_b[:], Exp, scale=-1.0)
                    for c2 in range(2):  # 1024-token chunks within b
                        t0c = b * T + c2 * 1024
                        bc = psw.tile([128, 1024], F32, tag="wei")
                        for n in range(2):
                            nc.tensor.matmul(
                                bc[:, ts(n, 512)],
                                hmask_sb[:],
                                recipSr_b[0:HPC, c2 * 1024 + n * 512:
                                          c2 * 1024 + (n + 1) * 512],
                                start=True, stop=True,
                            )
                        nc.vector.tensor_mul(
                            ycT[:, t0c:t0c + 1024], ycT[:, t0c:t0c + 1024], bc[:]
                        )
                    for tt in range(16):  # token tiles of this b
                        tok = b * T + tt * 128
                        op = psw.tile([128, 1024], F32, tag="wei")
                        for n in range(2):
                            nc.tensor.matmul(
                                op[:, ts(n, 512)],
                                ycT[:, tok:tok + 128],
                                wo_sb[:, ts(n, 512)],
                                start=True, stop=True,
                            )
                        y_sb = sby.tile([128, D], F32, tag="y")
                        nc.vector.tensor_copy(y_sb[:], op[:])
                        nc.sync.dma_start(
                            out=y_out[tok:tok + 128, :], in_=y_sb[:]
                        )

                    # ----- pass B (heads pair-interleaved): normalized att -----
                    for qt in range(NQT):
                        qs = b * T + qt * 128
                        att0 = sba.tile([128, T], F32, tag="att")
                        att1 = sba.tile([128, T], F32, tag="att")
                        for kc in range(2):  # 1024-wide k chunks
                            wei0 = psw.tile([128, 1024], F32, tag="wei")
                            wei1 = psw.tile([128, 1024], F32, tag="wei")
                            for n in range(2):
                                ks = b * T + kc * 1024 + n * 512
                                nc.tensor.matmul(
                                    wei0[:, ts(n, 512)],
                                    qT[0:HS, qs:qs + 128],
                                    kT[0:HS, ks:ks + 512],
                                    start=True, stop=True,
                                )
                                nc.tensor.matmul(
                                    wei1[:, ts(n, 512)],
                                    qT[HS:2 * HS, qs:qs + 128],
                                    kT[HS:2 * HS, ks:ks + 512],
                                    start=True, stop=True,
                                )
                            nc.scalar.activation(
                                att0[:, ts(kc, 1024)], wei0[:], Exp,
                                bias=negLnS[:, qt, b, 0:1],
                            )
                            nc.scalar.activation(
                                att1[:, ts(kc, 1024)], wei1[:], Exp,
                                bias=negLnS[:, qt, b, 1:2],
                            )
                        nc.sync.dma_start(
                            out=att_out[b, 0, ts(qt, 128), :], in_=att0[:]
                        )
                        nc.sync.dma_start(
                            out=att_out[b, 1, ts(qt, 128), :], in_=att1[:]
                        )

    nc.finalize()
    return nc


def kernel(x, Wq, Wk, Wv, Wo, bo, _trace=False, _tmpdir=None):
    global LAST_EXEC_NS, LAST_RESULT, _NC_CACHE
    x = np.asarray(x, dtype=np.float32)
    Wq = np.asarray(Wq, dtype=np.float32)
    Wk = np.asarray(Wk, dtype=np.float32)
    Wv = np.asarray(Wv, dtype=np.float32)
    Wo = np.asarray(Wo, dtype=np.float32)
    bo = np.asarray(bo, dtype=np.float32)

    scale = 1.0 / np.sqrt(np.float32(HS))
    xT_host = np.ascontiguousarray(x.reshape(BT, D).T)
    hm = np.zeros((HPC, 128), np.float32)
    for h in range(HPC):
        hm[h, h * HS:(h + 1) * HS] = 1.0

    in_maps = []
    for c in range(NCORES):
        cs = slice(c * CH, (c + 1) * CH)
        in_maps.append({
            "xT": xT_host,
            "wqT": np.ascontiguousarray(Wq[cs, :].T) * scale,
            "wkT": np.ascontiguousarray(Wk[cs, :].T),
            "wvT": np.ascontiguousarray(Wv[cs, :].T),
            "woT": np.ascontiguousarray(Wo[:, cs].T),
            "hmask": hm,
        })

    if _NC_CACHE is None:
        _NC_CACHE = _build()
    nc = _NC_CACHE

    res = run_bass_kernel_spmd(
        nc, in_maps, core_ids=list(range(NCORES)), trace=_trace, tmpdir=_tmpdir
    )
    LAST_EXEC_NS = res.exec_time_ns
    LAST_RESULT = res

    att = np.empty((B, H, T, T), dtype=np.float32)
    y = np.zeros((BT, D), dtype=np.float64)
    for c in range(NCORES):
        att[:, c * HPC:(c + 1) * HPC] = res.results[c]["att"]
        y += res.results[c]["y"]
    y = (y + bo).astype(np.float32).reshape(B, T, D)
    return y, att
